# revision 23
# baseline (speedup 1.0000x reference)
"""MultiHeadAttention Trainium2 kernel (8 NeuronCores).

Sharding: data-parallel over batch (2) x tensor-parallel over heads (16/4=4
head groups). Core c handles batch b = c//4 and heads 4g..4g+4 (g = c%4),
i.e. a 256-wide column slice of Wq/Wk/Wv and the matching row slice of Wo.
Each core computes a full [2048, 1024] partial output (its heads' ctx @ Wo
row-slice); the host sums the 4 partials per batch and adds the bias terms.

v4: fp16 on-chip datapath.  (fp8 Q/K + DoubleRow projections were built
and measured but rejected: e4m3 score noise passes through softmax to ctx
proportionally, ~9% max-err vs the 2e-2 gate.  fp8 DoublePixel scores are
numerically exact but run at half rate on this silicon.  A DVE Schraudolph
exp offload works (SCHR_KGS) but couples the DVE queue into the psA
critical path for no net gain, so it is off.)
On top of the v2 pipeline:
 - fp16 partial-output DMA (host sums partials in f32): halves out traffic.
 - head: K weights+bias DMA first, K chunk DMAs immediately after, other
   weights deferred -> first matmul starts ~11us in (was ~21us).
 - fine-grained schedule: V-proj/Q-proj/ctx/out-proj chains are emitted
   between scores kg-groups (PE program order = emission order), keeping
   the PE stream dense (>92% busy) while ACT drains psA score tiles.
 - V-chunk DMA prefetch ahead of the V-projection fillers; first K
   chunk split in two so the first matmul waits on only 0.75 MB of DMA;
   out-proj halves DMA'd as soon as each copy lands (shorter drain).
 - pT pool discipline: a scores call allocates 2 of 7 pT buffers; enough
   ctx consumers are emitted before the scores call that recycles their
   buffers (live pT <= 5 at every scores entry).
Measured: ~228us HW exec (baseline v2: ~243-250us), rel err 8.5e-4.
"""

import numpy as np

import concourse.bass as bass
import concourse.mybir as mybir
import concourse.tile as tile
from concourse import bacc
from concourse.bass_utils import run_bass_kernel_spmd

S = 2048          # sequence length
D = 1024          # model dim
DC = 256          # d' columns per core (4 heads x 64)
H = 4             # heads per core
DK = 64           # head dim
P = 128
F32 = mybir.dt.float32
FP16 = mybir.dt.float16
I16 = mybir.dt.int16
NCORES = 8

SCHR_KGS = ()     # kg tiles whose exp runs on DVE (Schraudolph)
WARMUP_MMS = 34   # dep-free 128-row matmuls that ramp the PE clock at t=0

# Schraudolph fp16-bit exp constants (scores arrive pre-scaled: the 0.125
# softmax scale is folded into Wq/bq on the host)
SCHR_C1 = 1024.0 * 1.4426950408889634
SCHR_C2 = 15360.0 - 46.0

_cached = {}


def build_program():
    nc = bacc.Bacc("TRN2", target_bir_lowering=False, debug=False,
                   num_devices=NCORES)

    # all staged inputs are host-pre-arranged so every DMA is one post with
    # per-partition-contiguous rows (128 fat descriptors, not 1024 thin ones)
    xqT = nc.dram_tensor("xqT", [P, 8 * S], FP16, kind="ExternalInput").ap()
    xkT = nc.dram_tensor("xkT", [P, 8 * S], FP16, kind="ExternalInput").ap()
    xvT = nc.dram_tensor("xvT", [P, 8 * S], FP16, kind="ExternalInput").ap()
    wqt = nc.dram_tensor("wqt", [P, 8 * DC], FP16, kind="ExternalInput").ap()
    wkt = nc.dram_tensor("wkt", [P, 8 * DC], FP16, kind="ExternalInput").ap()
    wvt = nc.dram_tensor("wvt", [P, 8 * DC], FP16, kind="ExternalInput").ap()
    wot = nc.dram_tensor("wot", [P, 2 * D], FP16, kind="ExternalInput").ap()
    bqr = nc.dram_tensor("bqr", [P, 2], F32, kind="ExternalInput").ap()
    bkr = nc.dram_tensor("bkr", [P, 2], F32, kind="ExternalInput").ap()
    out = nc.dram_tensor("out", [S, D], FP16, kind="ExternalOutput").ap()

    with tile.TileContext(nc) as tc:
        build_tile_kernel(nc, tc, xqT, xkT, xvT, wqt, wkt, wvt, wot,
                          bqr, bkr, out)

    nc.compile()
    return nc


def build_tile_kernel(nc, tc, xqT, xkT, xvT, wqt, wkt, wvt, wot,
                      bqr, bkr, out):
    from contextlib import ExitStack

    with ExitStack() as ctx:
        singles = ctx.enter_context(tc.tile_pool(name="singles", bufs=1))
        persist = ctx.enter_context(tc.tile_pool(name="persist", bufs=1))
        psA = ctx.enter_context(tc.tile_pool(name="psA", bufs=2, space="PSUM"))
        psB = ctx.enter_context(tc.tile_pool(name="psB", bufs=4, space="PSUM"))
        xT_pool = ctx.enter_context(tc.tile_pool(name="xT", bufs=3))
        pT_pool = ctx.enter_context(tc.tile_pool(name="pT", bufs=7))
        norm_pool = ctx.enter_context(tc.tile_pool(name="norm", bufs=3))
        out_sb_pool = ctx.enter_context(tc.tile_pool(name="osb", bufs=2))

        # --- PE warmup: dep-free dummy matmuls ramp the clock out of its low
        # p-state while the head DMAs land (real matmuls then start hot) -------
        warm = singles.tile([P, 128], FP16, tag="warm")
        nc.vector.memset(warm, 0.0)
        warm_ps = psB.tile([P, 128], F32, tag="ps1", name="warm_ps")
        for _ in range(WARMUP_MMS):
            nc.tensor.matmul(warm_ps, lhsT=warm, rhs=warm)

        # --- weights (K first so the K projection can start ASAP) ---------------
        w_k = singles.tile([P, 8, DC], FP16, tag="w_k")
        bk_t = singles.tile([P, 2], F32, tag="bk")
        # two posts: first matmul needs only dt 0-3; dt 4-7 drain behind it
        wk_r = wkt.rearrange("p (t c) -> p t c", c=DC)
        nc.sync.dma_start(out=w_k[:, 0:4, :], in_=wk_r[:, 0:4, :])

        # --- persistent activations ---------------------------------------------
        qT = persist.tile([P, 2, S], FP16, tag="qT")    # [d'%128, pair, s]
        kT = persist.tile([P, 2, S], FP16, tag="kT")
        v_sb = persist.tile([P, 16, H * (DK + 1)], FP16, tag="v_sb")
        ctxn = persist.tile([P, 2, S], FP16, tag="ctxn")  # [c%128, pair, q]

        # --- emit helpers ---------------------------------------------------------
        def emit_qk_proj(name, x_dram, w_t, b_t, dest, sc, xc=None):
            if xc is None:
                xc = xT_pool.tile([P, 8, 512], FP16, tag="xc",
                                  name=f"xc_{name}{sc}")
                nc.sync.dma_start(
                    out=xc,
                    in_=x_dram[:, 8 * 512 * sc:8 * 512 * (sc + 1)]
                        .rearrange("p (t s) -> p t s", s=512))
            for m in range(2):
                pr = psB.tile([P, 512], F32, tag="ps1",
                              name=f"pr_{name}_{sc}_{m}")
                for dt in range(8):
                    nc.tensor.matmul(
                        pr,
                        lhsT=w_t[:, dt, 128 * m:128 * (m + 1)],
                        rhs=xc[:, dt, :],
                        start=(dt == 0), stop=(dt == 7))
                nc.vector.tensor_scalar_add(
                    dest[:, m, 512 * sc:512 * (sc + 1)], pr, b_t[:, m:m + 1])

        xv_tiles = {}

        def vp_dma(sc):
            xc = xT_pool.tile([P, 8, 512], FP16, tag="xc", name=f"xv_{sc}")
            nc.sync.dma_start(
                out=xc,
                in_=xvT[:, 8 * 512 * sc:8 * 512 * (sc + 1)]
                    .rearrange("p (t s) -> p t s", s=512))
            xv_tiles[sc] = xc

        def emit_v_proj(sc):
            if sc not in xv_tiles:
                vp_dma(sc)
            xc = xv_tiles.pop(sc)
            for st in range(4):
                pv = psB.tile([P, DC], F32, tag="ps1", name=f"pv_{sc}_{st}")
                for dt in range(8):
                    nc.tensor.matmul(
                        pv,
                        lhsT=xc[:, dt, 128 * st:128 * (st + 1)],
                        rhs=w_v[:, dt, :],
                        start=(dt == 0), stop=(dt == 7))
                kt = 4 * sc + st
                nc.vector.tensor_copy(
                    v_sb[:, kt, :].rearrange("p (h c) -> p h c", h=H)[:, :, 0:DK],
                    pv.rearrange("p (h c) -> p h c", c=DK))

        pT_tiles = {}

        def emit_scores_exp(qc, pr_i, fillers=()):
            """Scores+exp for head pair pr_i over q-chunk qc.  `fillers` are
            emit-callbacks sprinkled between kg groups to keep the PE busy
            while ACT drains psA tiles."""
            fillers = list(fillers)
            qsl = slice(512 * qc, 512 * (qc + 1))
            h_a, h_b = 2 * pr_i, 2 * pr_i + 1
            pT_a = pT_pool.tile([P, 16, 512], FP16, tag="pT",
                                name=f"pT_{qc}_{h_a}")
            pT_b = pT_pool.tile([P, 16, 512], FP16, tag="pT",
                                name=f"pT_{qc}_{h_b}")
            pT_tiles[(qc, h_a)] = pT_a
            pT_tiles[(qc, h_b)] = pT_b
            for kg in range(8):
                sc_a = psA.tile([P, 2, 512], F32, tag="sc",
                                name=f"sca_{qc}_{pr_i}_{kg}")
                sc_b = psA.tile([P, 2, 512], F32, tag="sc",
                                name=f"scb_{qc}_{pr_i}_{kg}")
                for khi in range(2):
                    kt = 2 * kg + khi
                    ksl = slice(128 * kt, 128 * (kt + 1))
                    nc.tensor.matmul(sc_a[:, khi, :],
                                     lhsT=kT[0:64, pr_i, ksl],
                                     rhs=qT[0:64, pr_i, qsl])
                    nc.tensor.matmul(sc_b[:, khi, :],
                                     lhsT=kT[64:128, pr_i, ksl],
                                     rhs=qT[64:128, pr_i, qsl])
                for pt, sct in ((pT_a, sc_a), (pT_b, sc_b)):
                    dst = pt[:, 2 * kg:2 * kg + 2, :].rearrange(
                        "p a b -> p (a b)")
                    src = sct.rearrange("p a b -> p (a b)")
                    if kg in SCHR_KGS:
                        nc.vector.tensor_scalar(
                            out=dst.bitcast(I16), in0=src,
                            scalar1=SCHR_C1, scalar2=SCHR_C2,
                            op0=mybir.AluOpType.mult,
                            op1=mybir.AluOpType.add)
                    else:
                        nc.scalar.activation(
                            dst, src, mybir.ActivationFunctionType.Exp)
                if kg in (1, 3, 5) and fillers:
                    fillers.pop(0)()
            while fillers:
                fillers.pop(0)()

        def emit_ctx_norm(qc, h):
            qsl = slice(512 * qc, 512 * (qc + 1))
            pr_i, hp = divmod(h, 2)
            pT_h = pT_tiles.pop((qc, h))
            acc = psB.tile([P, 512], F32, tag="ps1", name=f"cp_{qc}_{h}")
            for kt in range(16):
                nc.tensor.matmul(
                    acc[0:65, :],
                    lhsT=v_sb[:, kt, 65 * h:65 * h + 65],
                    rhs=pT_h[:, kt, :],
                    start=(kt == 0), stop=(kt == 15))
            # normalize: ctx_n = ctx * broadcast(1/rowsum)
            rs = norm_pool.tile([1, 512], F32, tag="rs", name=f"rs_{qc}_{h}")
            nc.vector.tensor_copy(rs, acc[64:65, :])
            bc = norm_pool.tile([64, 512], F32, tag="bc", name=f"bc_{qc}_{h}")
            nc.gpsimd.partition_broadcast(bc, rs[0:1, :], channels=64)
            rc = norm_pool.tile([64, 512], F32, tag="rc", name=f"rc_{qc}_{h}")
            nc.vector.reciprocal_approx_fast(rc, bc)
            nc.vector.tensor_mul(
                ctxn[64 * hp:64 * hp + 64, pr_i, qsl], acc[0:64, :], rc)

        def emit_outproj_st(st):
            ob = out_sb_pool.tile([P, D], FP16, tag="ob", name=f"ob_{st}")
            for jc in range(2):
                op = psB.tile([P, 512], F32, tag="ps1", name=f"op_{st}_{jc}")
                for ct in range(2):
                    nc.tensor.matmul(
                        op,
                        lhsT=ctxn[:, ct, 128 * st:128 * (st + 1)],
                        rhs=w_o[:, ct, 512 * jc:512 * (jc + 1)],
                        start=(ct == 0), stop=(ct == 1))
                nc.vector.tensor_copy(ob[:, 512 * jc:512 * (jc + 1)], op)
            # one post per st: 128 contiguous 2KB rows in out
            nc.sync.dma_start(out=out[128 * st:128 * (st + 1), :], in_=ob)

        # --- emission schedule (software pipeline) --------------------------------
        def ctx_(qc, h):
            return lambda: emit_ctx_norm(qc, h)

        def op(st):
            return lambda: emit_outproj_st(st)

        def vp(sc):
            return lambda: emit_v_proj(sc)

        def emit_qk_proj_half(name, x_dram, w_t, b_t, dest, hc):
            xc = xT_pool.tile([P, 8, 256], FP16, tag="xc",
                              name=f"xh_{name}{hc}")
            nc.sync.dma_start(
                out=xc,
                in_=x_dram[:, 8 * 256 * hc:8 * 256 * (hc + 1)]
                    .rearrange("p (t s) -> p t s", s=256))
            for m in range(2):
                pr = psB.tile([P, 256], F32, tag="ps1",
                              name=f"prh_{name}_{hc}_{m}")
                for dt in range(8):
                    nc.tensor.matmul(
                        pr,
                        lhsT=w_t[:, dt, 128 * m:128 * (m + 1)],
                        rhs=xc[:, dt, :],
                        start=(dt == 0), stop=(dt == 7))
                nc.vector.tensor_scalar_add(
                    dest[:, m, 256 * hc:256 * (hc + 1)], pr, b_t[:, m:m + 1])

        emit_qk_proj_half("k", xkT, w_k, bk_t, kT, 0)
        nc.sync.dma_start(out=w_k[:, 4:8, :], in_=wk_r[:, 4:8, :])
        nc.sync.dma_start(out=bk_t, in_=bkr)
        emit_qk_proj_half("k", xkT, w_k, bk_t, kT, 1)
        for sc in range(1, 4):
            emit_qk_proj("k", xkT, w_k, bk_t, kT, sc)

        w_q = singles.tile([P, 8, DC], FP16, tag="w_q")
        bq_t = singles.tile([P, 2], F32, tag="bq")
        nc.sync.dma_start(out=w_q, in_=wqt.rearrange("p (t c) -> p t c", c=DC))
        nc.sync.dma_start(out=bq_t, in_=bqr)
        emit_qk_proj("q", xqT, w_q, bq_t, qT, 0)

        w_v = singles.tile([P, 8, DC], FP16, tag="w_v")
        nc.sync.dma_start(out=w_v, in_=wvt.rearrange("p (t c) -> p t c", c=DC))
        vp_dma(0)
        vp_dma(1)
        for h in range(H):  # ones column per head for rowsum-in-matmul
            nc.vector.memset(v_sb[:, :, h * 65 + 64:h * 65 + 65], 1.0)
        w_o = singles.tile([P, 2, D], FP16, tag="w_o")
        nc.sync.dma_start(out=w_o, in_=wot.rearrange("p (t j) -> p t j", j=D))

        xq_tiles = {}

        def qp_dma(sc):
            xc = xT_pool.tile([P, 8, 512], FP16, tag="xc", name=f"xq_{sc}")
            nc.sync.dma_start(
                out=xc,
                in_=xqT[:, 8 * 512 * sc:8 * 512 * (sc + 1)]
                    .rearrange("p (t s) -> p t s", s=512))
            xq_tiles[sc] = xc

        def emit_q_proj(sc):
            emit_qk_proj("q", xqT, w_q, bq_t, qT, sc, xc=xq_tiles.pop(sc))

        def qp(sc):
            return lambda: emit_q_proj(sc)

        # pT pool holds 7 tiles; a scores call allocates 2, so enough ctx
        # consumers must be emitted before the scores call that recycles
        # their buffers (live pT <= 5 at every scores entry).
        # x-chunk DMAs are prefetched 1+ scores-call ahead so filler
        # projections never wait on an in-flight post.
        emit_scores_exp(0, 0, fillers=[vp(0), vp(1)])
        vp_dma(2)
        vp_dma(3)
        qp_dma(1)
        emit_scores_exp(0, 1, fillers=[vp(2), vp(3)])
        emit_q_proj(1)
        qp_dma(2)
        emit_scores_exp(1, 0, fillers=[qp(2)])
        qp_dma(3)
        emit_ctx_norm(0, 0)
        emit_ctx_norm(0, 1)
        emit_scores_exp(1, 1, fillers=[qp(3), ctx_(0, 2)])
        emit_scores_exp(2, 0, fillers=[ctx_(0, 3), op(0), op(1)])
        emit_ctx_norm(1, 0)
        emit_ctx_norm(1, 1)
        emit_scores_exp(2, 1, fillers=[op(2), op(3), ctx_(1, 2)])
        emit_scores_exp(3, 0, fillers=[ctx_(1, 3), op(4), op(5)])
        emit_ctx_norm(2, 0)
        emit_ctx_norm(2, 1)
        emit_scores_exp(3, 1, fillers=[op(6), op(7), ctx_(2, 2), ctx_(2, 3),
                                       op(8), op(9), ctx_(3, 0), ctx_(3, 1)])
        emit_outproj_st(10)
        emit_outproj_st(11)
        emit_ctx_norm(3, 2)
        emit_ctx_norm(3, 3)
        for st in range(12, 16):
            emit_outproj_st(st)


def _stage_x(xT, widths):
    """xT [D, S] -> [128, 8*S]: consumption-ordered col blocks, each stored
    p-major so every chunk DMA is 128 contiguous per-partition rows."""
    blocks = []
    c0 = 0
    for w in widths:
        blk = xT[:, c0:c0 + w].reshape(8, P, w).transpose(1, 0, 2)
        blocks.append(blk.reshape(P, 8 * w))
        c0 += w
    return np.ascontiguousarray(np.concatenate(blocks, axis=1))


def _stage_w(wt):
    """wt [D, DC] -> [128, 8*DC] p-major (tile layout [P, 8, DC] flattened)."""
    return np.ascontiguousarray(
        wt.reshape(8, P, DC).transpose(1, 0, 2).reshape(P, 8 * DC))


K_WIDTHS = (256, 256, 512, 512, 512)
QV_WIDTHS = (512, 512, 512, 512)


def make_in_maps(Q_input, K_input, V_input, Wq, bq, Wk, bk, Wv, Wo):
    scale = 0.125  # 1/sqrt(64), exact power of two
    xS = {}
    for b in range(2):
        xS[("q", b)] = _stage_x(Q_input[b].T.astype(np.float16), QV_WIDTHS)
        xS[("k", b)] = _stage_x(K_input[b].T.astype(np.float16), K_WIDTHS)
        xS[("v", b)] = _stage_x(V_input[b].T.astype(np.float16), QV_WIDTHS)
    in_maps = []
    for c in range(NCORES):
        b, g = divmod(c, 4)
        sl = slice(DC * g, DC * (g + 1))
        wo_p = np.ascontiguousarray(
            Wo[:, sl].T.reshape(2, P, D).transpose(1, 0, 2).reshape(P, 2 * D))
        in_maps.append({
            "xqT": xS[("q", b)],
            "xkT": xS[("k", b)],
            "xvT": xS[("v", b)],
            "wqt": _stage_w((Wq[sl, :].T * scale).astype(np.float16)),
            "wkt": _stage_w(Wk[sl, :].T.astype(np.float16)),
            "wvt": _stage_w(Wv[sl, :].T.astype(np.float16)),
            "wot": wo_p.astype(np.float16),
            "bqr": (bq[sl] * scale).reshape(2, P).T.astype(np.float32),
            "bkr": bk[sl].reshape(2, P).T.astype(np.float32),
        })
    return in_maps


def kernel(Q_input, K_input, V_input, Wq, bq, Wk, bk, Wv, bv, Wo, bo):
    if "nc" not in _cached:
        _cached["nc"] = build_program()
    nc = _cached["nc"]

    in_maps = make_in_maps(Q_input, K_input, V_input, Wq, bq, Wk, bk, Wv, Wo)
    res = run_bass_kernel_spmd(nc, in_maps, list(range(NCORES))).results
    outs = [res[c]["out"] for c in range(NCORES)]

    const = (bv.astype(np.float32) @ Wo.T.astype(np.float32)) + bo
    full = np.empty((2, S, D), np.float32)
    for b in range(2):
        acc = outs[4 * b].astype(np.float32)
        for g in range(1, 4):
            acc += outs[4 * b + g]
        full[b] = acc + const
    return full



# revision 32
# speedup vs baseline: 1.0211x; 1.0211x over previous
"""MultiHeadAttention Trainium2 kernel (8 NeuronCores).

Sharding: data-parallel over batch (2) x tensor-parallel over heads (16/4=4
head groups). Core c handles batch b = c//4 and heads 4g..4g+4 (g = c%4),
i.e. a 256-wide column slice of Wq/Wk/Wv and the matching row slice of Wo.
Each core computes a full [2048, 1024] partial output (its heads' ctx @ Wo
row-slice); the host sums the 4 partials per batch and adds the bias terms.

v4: fp16 on-chip datapath.  (fp8 Q/K + DoubleRow projections were built
and measured but rejected: e4m3 score noise passes through softmax to ctx
proportionally, ~9% max-err vs the 2e-2 gate.  fp8 DoublePixel scores are
numerically exact but run at half rate on this silicon.  A DVE Schraudolph
exp offload works (SCHR_KGS) but couples the DVE queue into the psA
critical path for no net gain, so it is off.)
On top of the v2 pipeline:
 - fp16 partial-output DMA (host sums partials in f32): halves out traffic.
 - head: K weights+bias DMA first, K chunk DMAs immediately after, other
   weights deferred -> first matmul starts ~11us in (was ~21us).
 - fine-grained schedule: V-proj/Q-proj/ctx/out-proj chains are emitted
   between scores kg-groups (PE program order = emission order), keeping
   the PE stream dense (>92% busy) while ACT drains psA score tiles.
 - V-chunk DMA prefetch ahead of the V-projection fillers; first K
   chunk split in two so the first matmul waits on only 0.75 MB of DMA;
   out-proj halves DMA'd as soon as each copy lands (shorter drain).
 - pT pool discipline: a scores call allocates 2 of 7 pT buffers; enough
   ctx consumers are emitted before the scores call that recycles their
   buffers (live pT <= 5 at every scores entry).
Measured: ~228us HW exec (baseline v2: ~243-250us), rel err 8.5e-4.
"""

import numpy as np

import concourse.bass as bass
import concourse.mybir as mybir
import concourse.tile as tile
from concourse import bacc
from concourse.bass_utils import run_bass_kernel_spmd

S = 2048          # sequence length
D = 1024          # model dim
DC = 256          # d' columns per core (4 heads x 64)
H = 4             # heads per core
DK = 64           # head dim
P = 128
F32 = mybir.dt.float32
FP16 = mybir.dt.float16
I16 = mybir.dt.int16
NCORES = 8

SCHR_KGS = ()     # kg tiles whose exp runs on DVE (Schraudolph)
WARMUP_MMS = 34   # dep-free 128-row matmuls that ramp the PE clock at t=0

# Schraudolph fp16-bit exp constants (scores arrive pre-scaled: the 0.125
# softmax scale is folded into Wq/bq on the host)
SCHR_C1 = 1024.0 * 1.4426950408889634
SCHR_C2 = 15360.0 - 46.0

_cached = {}


def build_program():
    nc = bacc.Bacc("TRN2", target_bir_lowering=False, debug=False,
                   num_devices=NCORES)

    # all staged inputs are host-pre-arranged so every DMA is one post with
    # per-partition-contiguous rows (128 fat descriptors, not 1024 thin ones)
    xqT = nc.dram_tensor("xqT", [P, 8 * S], FP16, kind="ExternalInput").ap()
    xkT = nc.dram_tensor("xkT", [P, 8 * S], FP16, kind="ExternalInput").ap()
    xvT = nc.dram_tensor("xvT", [P, 8 * S], FP16, kind="ExternalInput").ap()
    wqt = nc.dram_tensor("wqt", [P, 8 * DC], FP16, kind="ExternalInput").ap()
    wkt = nc.dram_tensor("wkt", [P, 8 * DC], FP16, kind="ExternalInput").ap()
    wvt = nc.dram_tensor("wvt", [P, 8 * DC], FP16, kind="ExternalInput").ap()
    wot = nc.dram_tensor("wot", [P, 2 * D], FP16, kind="ExternalInput").ap()
    bqr = nc.dram_tensor("bqr", [P, 2], F32, kind="ExternalInput").ap()
    bkr = nc.dram_tensor("bkr", [P, 2], F32, kind="ExternalInput").ap()
    out = nc.dram_tensor("out", [S, D], FP16, kind="ExternalOutput").ap()

    with tile.TileContext(nc) as tc:
        build_tile_kernel(nc, tc, xqT, xkT, xvT, wqt, wkt, wvt, wot,
                          bqr, bkr, out)

    nc.compile()
    return nc


def build_tile_kernel(nc, tc, xqT, xkT, xvT, wqt, wkt, wvt, wot,
                      bqr, bkr, out):
    from contextlib import ExitStack

    with ExitStack() as ctx:
        singles = ctx.enter_context(tc.tile_pool(name="singles", bufs=1))
        persist = ctx.enter_context(tc.tile_pool(name="persist", bufs=1))
        psA = ctx.enter_context(tc.tile_pool(name="psA", bufs=2, space="PSUM"))
        psB = ctx.enter_context(tc.tile_pool(name="psB", bufs=4, space="PSUM"))
        xT_pool = ctx.enter_context(tc.tile_pool(name="xT", bufs=4))
        pT_pool = ctx.enter_context(tc.tile_pool(name="pT", bufs=7))
        norm_pool = ctx.enter_context(tc.tile_pool(name="norm", bufs=2))
        out_sb_pool = ctx.enter_context(tc.tile_pool(name="osb", bufs=2))

        # --- PE warmup: dep-free dummy matmuls ramp the clock out of its low
        # p-state while the head DMAs land (real matmuls then start hot) -------
        warm = singles.tile([P, 128], FP16, tag="warm")
        nc.vector.memset(warm, 0.0)
        warm_ps = psB.tile([P, 128], F32, tag="ps1", name="warm_ps")
        for _ in range(WARMUP_MMS):
            nc.tensor.matmul(warm_ps, lhsT=warm, rhs=warm)

        # --- weights (K first so the K projection can start ASAP) ---------------
        w_k = singles.tile([P, 8, DC], FP16, tag="w_k")
        bk_t = singles.tile([P, 2], F32, tag="bk")
        # two posts: first matmul needs only dt 0-3; dt 4-7 drain behind it
        wk_r = wkt.rearrange("p (t c) -> p t c", c=DC)
        nc.sync.dma_start(out=w_k[:, 0:4, :], in_=wk_r[:, 0:4, :])

        # --- persistent activations ---------------------------------------------
        qT = persist.tile([P, 2, S], FP16, tag="qT")    # [d'%128, pair, s]
        kT = persist.tile([P, 2, S], FP16, tag="kT")
        v_sb = persist.tile([P, 16, H * (DK + 1)], FP16, tag="v_sb")
        ctxn = persist.tile([P, 2, S], FP16, tag="ctxn")  # [c%128, pair, q]

        # --- emit helpers ---------------------------------------------------------
        def emit_qk_proj(name, x_dram, w_t, b_t, dest, sc, xc=None):
            if xc is None:
                xc = xT_pool.tile([P, 8, 512], FP16, tag="xc",
                                  name=f"xc_{name}{sc}")
                nc.sync.dma_start(
                    out=xc,
                    in_=x_dram[:, 8 * 512 * sc:8 * 512 * (sc + 1)]
                        .rearrange("p (t s) -> p t s", s=512))
            for m in range(2):
                pr = psB.tile([P, 512], F32, tag="ps1",
                              name=f"pr_{name}_{sc}_{m}")
                for dt in range(8):
                    nc.tensor.matmul(
                        pr,
                        lhsT=w_t[:, dt, 128 * m:128 * (m + 1)],
                        rhs=xc[:, dt, :],
                        start=(dt == 0), stop=(dt == 7))
                nc.vector.tensor_scalar_add(
                    dest[:, m, 512 * sc:512 * (sc + 1)], pr, b_t[:, m:m + 1])

        xv_tiles = {}

        def vp_dma(sc):
            xc = xT_pool.tile([P, 8, 512], FP16, tag="xc", name=f"xv_{sc}")
            nc.sync.dma_start(
                out=xc,
                in_=xvT[:, 8 * 512 * sc:8 * 512 * (sc + 1)]
                    .rearrange("p (t s) -> p t s", s=512))
            xv_tiles[sc] = xc

        def emit_v_proj(sc):
            if sc not in xv_tiles:
                vp_dma(sc)
            xc = xv_tiles.pop(sc)
            for st in range(4):
                pv = psB.tile([P, DC], F32, tag="ps1", name=f"pv_{sc}_{st}")
                for dt in range(8):
                    nc.tensor.matmul(
                        pv,
                        lhsT=xc[:, dt, 128 * st:128 * (st + 1)],
                        rhs=w_v[:, dt, :],
                        start=(dt == 0), stop=(dt == 7))
                kt = 4 * sc + st
                nc.vector.tensor_copy(
                    v_sb[:, kt, :].rearrange("p (h c) -> p h c", h=H)[:, :, 0:DK],
                    pv.rearrange("p (h c) -> p h c", c=DK))

        pT_tiles = {}

        def emit_scores_exp(qc, pr_i, fillers=()):
            """Scores+exp for head pair pr_i over q-chunk qc.  `fillers` are
            emit-callbacks sprinkled between kg groups to keep the PE busy
            while ACT drains psA tiles."""
            fillers = list(fillers)
            qsl = slice(512 * qc, 512 * (qc + 1))
            h_a, h_b = 2 * pr_i, 2 * pr_i + 1
            pT_a = pT_pool.tile([P, 16, 512], FP16, tag="pT",
                                name=f"pT_{qc}_{h_a}")
            pT_b = pT_pool.tile([P, 16, 512], FP16, tag="pT",
                                name=f"pT_{qc}_{h_b}")
            pT_tiles[(qc, h_a)] = pT_a
            pT_tiles[(qc, h_b)] = pT_b
            for kg in range(8):
                sc_a = psA.tile([P, 2, 512], F32, tag="sc",
                                name=f"sca_{qc}_{pr_i}_{kg}")
                sc_b = psA.tile([P, 2, 512], F32, tag="sc",
                                name=f"scb_{qc}_{pr_i}_{kg}")
                for khi in range(2):
                    kt = 2 * kg + khi
                    ksl = slice(128 * kt, 128 * (kt + 1))
                    nc.tensor.matmul(sc_a[:, khi, :],
                                     lhsT=kT[0:64, pr_i, ksl],
                                     rhs=qT[0:64, pr_i, qsl])
                    nc.tensor.matmul(sc_b[:, khi, :],
                                     lhsT=kT[64:128, pr_i, ksl],
                                     rhs=qT[64:128, pr_i, qsl])
                for pt, sct in ((pT_a, sc_a), (pT_b, sc_b)):
                    dst = pt[:, 2 * kg:2 * kg + 2, :].rearrange(
                        "p a b -> p (a b)")
                    src = sct.rearrange("p a b -> p (a b)")
                    if kg in SCHR_KGS:
                        nc.vector.tensor_scalar(
                            out=dst.bitcast(I16), in0=src,
                            scalar1=SCHR_C1, scalar2=SCHR_C2,
                            op0=mybir.AluOpType.mult,
                            op1=mybir.AluOpType.add)
                    else:
                        nc.scalar.activation(
                            dst, src, mybir.ActivationFunctionType.Exp)
                if kg in (1, 3, 5) and fillers:
                    fillers.pop(0)()
            while fillers:
                fillers.pop(0)()

        def emit_ctx_norm(qc, h):
            qsl = slice(512 * qc, 512 * (qc + 1))
            pr_i, hp = divmod(h, 2)
            pT_h = pT_tiles.pop((qc, h))
            acc = psB.tile([P, 512], F32, tag="ps1", name=f"cp_{qc}_{h}")
            for kt in range(16):
                nc.tensor.matmul(
                    acc[0:65, :],
                    lhsT=v_sb[:, kt, 65 * h:65 * h + 65],
                    rhs=pT_h[:, kt, :],
                    start=(kt == 0), stop=(kt == 15))
            # normalize: ctx_n = ctx * broadcast(1/rowsum)
            rs = norm_pool.tile([1, 512], F32, tag="rs", name=f"rs_{qc}_{h}")
            nc.vector.tensor_copy(rs, acc[64:65, :])
            bc = norm_pool.tile([64, 512], F32, tag="bc", name=f"bc_{qc}_{h}")
            nc.gpsimd.partition_broadcast(bc, rs[0:1, :], channels=64)
            nc.vector.reciprocal_approx_fast(bc, bc)
            nc.vector.tensor_mul(
                ctxn[64 * hp:64 * hp + 64, pr_i, qsl], acc[0:64, :], bc)

        def emit_outproj_st(st):
            ob = out_sb_pool.tile([P, D], FP16, tag="ob", name=f"ob_{st}")
            for jc in range(2):
                op = psB.tile([P, 512], F32, tag="ps1", name=f"op_{st}_{jc}")
                for ct in range(2):
                    nc.tensor.matmul(
                        op,
                        lhsT=ctxn[:, ct, 128 * st:128 * (st + 1)],
                        rhs=w_o[:, ct, 512 * jc:512 * (jc + 1)],
                        start=(ct == 0), stop=(ct == 1))
                nc.vector.tensor_copy(ob[:, 512 * jc:512 * (jc + 1)], op)
            # one post per st: 128 contiguous 2KB rows in out
            nc.sync.dma_start(out=out[128 * st:128 * (st + 1), :], in_=ob)

        # --- emission schedule (software pipeline) --------------------------------
        def ctx_(qc, h):
            return lambda: emit_ctx_norm(qc, h)

        def op(st):
            return lambda: emit_outproj_st(st)

        def vp(sc):
            return lambda: emit_v_proj(sc)

        def chunk_dma(x_dram, c0, cols, name):
            xc = xT_pool.tile([P, 8, cols], FP16, tag="xc",
                              name=f"xh_{name}{c0}")
            nc.sync.dma_start(
                out=xc,
                in_=x_dram[:, 8 * c0:8 * (c0 + cols)]
                    .rearrange("p (t s) -> p t s", s=cols))
            return xc

        def emit_qk_proj_chunk(name, xc, w_t, b_t, dest, c0, cols):
            for m in range(2):
                pr = psB.tile([P, cols], F32, tag="ps1",
                              name=f"prh_{name}_{c0}_{m}")
                for dt in range(8):
                    nc.tensor.matmul(
                        pr,
                        lhsT=w_t[:, dt, 128 * m:128 * (m + 1)],
                        rhs=xc[:, dt, :],
                        start=(dt == 0), stop=(dt == 7))
                nc.vector.tensor_scalar_add(
                    dest[:, m, c0:c0 + cols], pr, b_t[:, m:m + 1])

        # post order is emission order on the Sync engine; every post is
        # emitted BEFORE any instruction that reads its tile (Tile deps are
        # emission-order-based), with the critical first chunks up front.
        xkA = chunk_dma(xkT, 0, 128, "k")
        nc.sync.dma_start(out=w_k[:, 4:8, :], in_=wk_r[:, 4:8, :])
        nc.sync.dma_start(out=bk_t, in_=bkr)
        xkB = chunk_dma(xkT, 128, 128, "k")
        emit_qk_proj_chunk("k", xkA, w_k, bk_t, kT, 0, 128)
        xkC = chunk_dma(xkT, 256, 256, "k")
        emit_qk_proj_chunk("k", xkB, w_k, bk_t, kT, 128, 128)
        emit_qk_proj_chunk("k", xkC, w_k, bk_t, kT, 256, 256)
        for sc in range(1, 4):
            emit_qk_proj("k", xkT, w_k, bk_t, kT, sc)

        w_q = singles.tile([P, 8, DC], FP16, tag="w_q")
        bq_t = singles.tile([P, 2], F32, tag="bq")
        nc.sync.dma_start(out=w_q, in_=wqt.rearrange("p (t c) -> p t c", c=DC))
        nc.sync.dma_start(out=bq_t, in_=bqr)
        emit_qk_proj("q", xqT, w_q, bq_t, qT, 0)

        w_v = singles.tile([P, 8, DC], FP16, tag="w_v")
        nc.sync.dma_start(out=w_v, in_=wvt.rearrange("p (t c) -> p t c", c=DC))
        vp_dma(0)
        vp_dma(1)
        for h in range(H):  # ones column per head for rowsum-in-matmul
            nc.vector.memset(v_sb[:, :, h * 65 + 64:h * 65 + 65], 1.0)
        w_o = singles.tile([P, 2, D], FP16, tag="w_o")
        nc.sync.dma_start(out=w_o, in_=wot.rearrange("p (t j) -> p t j", j=D))

        xq_tiles = {}

        def qp_dma(sc):
            xc = xT_pool.tile([P, 8, 512], FP16, tag="xc", name=f"xq_{sc}")
            nc.sync.dma_start(
                out=xc,
                in_=xqT[:, 8 * 512 * sc:8 * 512 * (sc + 1)]
                    .rearrange("p (t s) -> p t s", s=512))
            xq_tiles[sc] = xc

        def emit_q_proj(sc):
            emit_qk_proj("q", xqT, w_q, bq_t, qT, sc, xc=xq_tiles.pop(sc))

        def qp(sc):
            return lambda: emit_q_proj(sc)

        # pT pool holds 7 tiles; a scores call allocates 2, so enough ctx
        # consumers must be emitted before the scores call that recycles
        # their buffers (live pT <= 5 at every scores entry).
        # x-chunk DMAs are prefetched 1+ scores-call ahead so filler
        # projections never wait on an in-flight post.
        emit_scores_exp(0, 0, fillers=[vp(0), vp(1)])
        vp_dma(2)
        vp_dma(3)
        qp_dma(1)
        emit_scores_exp(0, 1, fillers=[vp(2), vp(3)])
        emit_q_proj(1)
        qp_dma(2)
        emit_scores_exp(1, 0, fillers=[qp(2)])
        qp_dma(3)
        emit_ctx_norm(0, 0)
        emit_ctx_norm(0, 1)
        emit_scores_exp(1, 1, fillers=[qp(3), ctx_(0, 2)])
        emit_scores_exp(2, 0, fillers=[ctx_(0, 3), op(0), op(1)])
        emit_ctx_norm(1, 0)
        emit_ctx_norm(1, 1)
        emit_scores_exp(2, 1, fillers=[op(2), op(3), ctx_(1, 2)])
        emit_scores_exp(3, 0, fillers=[ctx_(1, 3), op(4), op(5)])
        emit_ctx_norm(2, 0)
        emit_ctx_norm(2, 1)
        emit_scores_exp(3, 1, fillers=[op(6), op(7), ctx_(2, 2), ctx_(2, 3),
                                       op(8), op(9), ctx_(3, 0), ctx_(3, 1)])
        emit_outproj_st(10)
        emit_outproj_st(11)
        emit_ctx_norm(3, 2)
        emit_ctx_norm(3, 3)
        for st in range(12, 16):
            emit_outproj_st(st)


def _stage_x(xT, widths):
    """xT [D, S] -> [128, 8*S]: consumption-ordered col blocks, each stored
    p-major so every chunk DMA is 128 contiguous per-partition rows."""
    blocks = []
    c0 = 0
    for w in widths:
        blk = xT[:, c0:c0 + w].reshape(8, P, w).transpose(1, 0, 2)
        blocks.append(blk.reshape(P, 8 * w))
        c0 += w
    return np.ascontiguousarray(np.concatenate(blocks, axis=1))


def _stage_w(wt):
    """wt [D, DC] -> [128, 8*DC] p-major (tile layout [P, 8, DC] flattened)."""
    return np.ascontiguousarray(
        wt.reshape(8, P, DC).transpose(1, 0, 2).reshape(P, 8 * DC))


K_WIDTHS = (128, 128, 256, 512, 512, 512)
QV_WIDTHS = (512, 512, 512, 512)


def make_in_maps(Q_input, K_input, V_input, Wq, bq, Wk, bk, Wv, Wo):
    scale = 0.125  # 1/sqrt(64), exact power of two
    xS = {}
    for b in range(2):
        xS[("q", b)] = _stage_x(Q_input[b].T.astype(np.float16), QV_WIDTHS)
        xS[("k", b)] = _stage_x(K_input[b].T.astype(np.float16), K_WIDTHS)
        xS[("v", b)] = _stage_x(V_input[b].T.astype(np.float16), QV_WIDTHS)
    in_maps = []
    for c in range(NCORES):
        b, g = divmod(c, 4)
        sl = slice(DC * g, DC * (g + 1))
        wo_p = np.ascontiguousarray(
            Wo[:, sl].T.reshape(2, P, D).transpose(1, 0, 2).reshape(P, 2 * D))
        in_maps.append({
            "xqT": xS[("q", b)],
            "xkT": xS[("k", b)],
            "xvT": xS[("v", b)],
            "wqt": _stage_w((Wq[sl, :].T * scale).astype(np.float16)),
            "wkt": _stage_w(Wk[sl, :].T.astype(np.float16)),
            "wvt": _stage_w(Wv[sl, :].T.astype(np.float16)),
            "wot": wo_p.astype(np.float16),
            "bqr": (bq[sl] * scale).reshape(2, P).T.astype(np.float32),
            "bkr": bk[sl].reshape(2, P).T.astype(np.float32),
        })
    return in_maps


def kernel(Q_input, K_input, V_input, Wq, bq, Wk, bk, Wv, bv, Wo, bo):
    if "nc" not in _cached:
        _cached["nc"] = build_program()
    nc = _cached["nc"]

    in_maps = make_in_maps(Q_input, K_input, V_input, Wq, bq, Wk, bk, Wv, Wo)
    res = run_bass_kernel_spmd(nc, in_maps, list(range(NCORES))).results
    outs = [res[c]["out"] for c in range(NCORES)]

    const = (bv.astype(np.float32) @ Wo.T.astype(np.float32)) + bo
    full = np.empty((2, S, D), np.float32)
    for b in range(2):
        acc = outs[4 * b].astype(np.float32)
        for g in range(1, 4):
            acc += outs[4 * b + g]
        full[b] = acc + const
    return full



# revision 35
# speedup vs baseline: 1.0327x; 1.0114x over previous
"""MultiHeadAttention Trainium2 kernel (8 NeuronCores).

Sharding: data-parallel over batch (2) x tensor-parallel over heads (16/4=4
head groups). Core c handles batch b = c//4 and heads 4g..4g+4 (g = c%4),
i.e. a 256-wide column slice of Wq/Wk/Wv and the matching row slice of Wo.
Each core computes a full [2048, 1024] partial output (its heads' ctx @ Wo
row-slice); the host sums the 4 partials per batch and adds the bias terms.

v4: fp16 on-chip datapath.  (fp8 Q/K + DoubleRow projections were built
and measured but rejected: e4m3 score noise passes through softmax to ctx
proportionally, ~9% max-err vs the 2e-2 gate.  fp8 DoublePixel scores are
numerically exact but run at half rate on this silicon.  A DVE Schraudolph
exp offload works (SCHR_KGS) but couples the DVE queue into the psA
critical path for no net gain, so it is off.)
On top of the v2 pipeline:
 - fp16 partial-output DMA (host sums partials in f32): halves out traffic.
 - head: K weights+bias DMA first, K chunk DMAs immediately after, other
   weights deferred -> first matmul starts ~11us in (was ~21us).
 - fine-grained schedule: V-proj/Q-proj/ctx/out-proj chains are emitted
   between scores kg-groups (PE program order = emission order), keeping
   the PE stream dense (>92% busy) while ACT drains psA score tiles.
 - V-chunk DMA prefetch ahead of the V-projection fillers; first K
   chunk split in two so the first matmul waits on only 0.75 MB of DMA;
   out-proj halves DMA'd as soon as each copy lands (shorter drain).
 - pT pool discipline: a scores call allocates 2 of 7 pT buffers; enough
   ctx consumers are emitted before the scores call that recycles their
   buffers (live pT <= 5 at every scores entry).
Measured: ~228us HW exec (baseline v2: ~243-250us), rel err 8.5e-4.
"""

import numpy as np

import concourse.bass as bass
import concourse.mybir as mybir
import concourse.tile as tile
from concourse import bacc
from concourse.bass_utils import run_bass_kernel_spmd

S = 2048          # sequence length
D = 1024          # model dim
DC = 256          # d' columns per core (4 heads x 64)
H = 4             # heads per core
DK = 64           # head dim
P = 128
F32 = mybir.dt.float32
FP16 = mybir.dt.float16
I16 = mybir.dt.int16
NCORES = 8

SCHR_KGS = ()     # kg tiles whose exp runs on DVE (Schraudolph)
WARMUP_MMS = 34   # dep-free 128-row matmuls that ramp the PE clock at t=0

# Schraudolph fp16-bit exp constants (scores arrive pre-scaled: the 0.125
# softmax scale is folded into Wq/bq on the host)
SCHR_C1 = 1024.0 * 1.4426950408889634
SCHR_C2 = 15360.0 - 46.0

_cached = {}


def build_program():
    nc = bacc.Bacc("TRN2", target_bir_lowering=False, debug=False,
                   num_devices=NCORES)

    # all staged inputs are host-pre-arranged so every DMA is one post with
    # per-partition-contiguous rows (128 fat descriptors, not 1024 thin ones)
    xqT = nc.dram_tensor("xqT", [P, 8 * S], FP16, kind="ExternalInput").ap()
    xkT = nc.dram_tensor("xkT", [P, 8 * S], FP16, kind="ExternalInput").ap()
    xvT = nc.dram_tensor("xvT", [P, 8 * S], FP16, kind="ExternalInput").ap()
    wqt = nc.dram_tensor("wqt", [P, 8 * DC], FP16, kind="ExternalInput").ap()
    wkt = nc.dram_tensor("wkt", [P, 8 * DC], FP16, kind="ExternalInput").ap()
    wvt = nc.dram_tensor("wvt", [P, 8 * DC], FP16, kind="ExternalInput").ap()
    wot = nc.dram_tensor("wot", [P, 2 * D], FP16, kind="ExternalInput").ap()
    bqr = nc.dram_tensor("bqr", [P, 2], F32, kind="ExternalInput").ap()
    bkr = nc.dram_tensor("bkr", [P, 2], F32, kind="ExternalInput").ap()
    out = nc.dram_tensor("out", [S, D], FP16, kind="ExternalOutput").ap()

    with tile.TileContext(nc) as tc:
        build_tile_kernel(nc, tc, xqT, xkT, xvT, wqt, wkt, wvt, wot,
                          bqr, bkr, out)

    nc.compile()
    return nc


def build_tile_kernel(nc, tc, xqT, xkT, xvT, wqt, wkt, wvt, wot,
                      bqr, bkr, out):
    from contextlib import ExitStack

    with ExitStack() as ctx:
        singles = ctx.enter_context(tc.tile_pool(name="singles", bufs=1))
        persist = ctx.enter_context(tc.tile_pool(name="persist", bufs=1))
        psA = ctx.enter_context(tc.tile_pool(name="psA", bufs=2, space="PSUM"))
        psB = ctx.enter_context(tc.tile_pool(name="psB", bufs=4, space="PSUM"))
        xT_pool = ctx.enter_context(tc.tile_pool(name="xT", bufs=4))
        pT_pool = ctx.enter_context(tc.tile_pool(name="pT", bufs=7))
        norm_pool = ctx.enter_context(tc.tile_pool(name="norm", bufs=2))
        out_sb_pool = ctx.enter_context(tc.tile_pool(name="osb", bufs=2))

        # --- PE warmup: dep-free dummy matmuls ramp the clock out of its low
        # p-state while the head DMAs land (real matmuls then start hot) -------
        warm = singles.tile([P, 128], FP16, tag="warm")
        nc.vector.memset(warm, 0.0)
        warm_ps = psB.tile([P, 128], F32, tag="ps1", name="warm_ps")
        for _ in range(WARMUP_MMS):
            nc.tensor.matmul(warm_ps, lhsT=warm, rhs=warm)

        # --- weights (K first so the K projection can start ASAP) ---------------
        w_k = singles.tile([P, 8, DC], FP16, tag="w_k")
        bk_t = singles.tile([P, 2], F32, tag="bk")
        # two posts: first matmul needs only dt 0-3; dt 4-7 drain behind it
        wk_r = wkt.rearrange("p (t c) -> p t c", c=DC)
        nc.sync.dma_start(out=w_k[:, 0:4, :], in_=wk_r[:, 0:4, :])

        # --- persistent activations ---------------------------------------------
        qT = persist.tile([P, 2, S], FP16, tag="qT")    # [d'%128, pair, s]
        kT = persist.tile([P, 2, S], FP16, tag="kT")
        v_sb = persist.tile([P, 16, H * (DK + 1)], FP16, tag="v_sb")
        ctxn = persist.tile([P, 2, S], FP16, tag="ctxn")  # [c%128, pair, q]

        # --- emit helpers ---------------------------------------------------------
        def emit_qk_proj(name, x_dram, w_t, b_t, dest, sc, xc=None):
            if xc is None:
                xc = xT_pool.tile([P, 8, 512], FP16, tag="xc",
                                  name=f"xc_{name}{sc}")
                nc.sync.dma_start(
                    out=xc,
                    in_=x_dram[:, 8 * 512 * sc:8 * 512 * (sc + 1)]
                        .rearrange("p (t s) -> p t s", s=512))
            for m in range(2):
                pr = psB.tile([P, 512], F32, tag="ps1",
                              name=f"pr_{name}_{sc}_{m}")
                for dt in range(8):
                    nc.tensor.matmul(
                        pr,
                        lhsT=w_t[:, dt, 128 * m:128 * (m + 1)],
                        rhs=xc[:, dt, :],
                        start=(dt == 0), stop=(dt == 7))
                nc.vector.tensor_scalar_add(
                    dest[:, m, 512 * sc:512 * (sc + 1)], pr, b_t[:, m:m + 1])

        xv_tiles = {}

        def vp_dma(sc):
            xc = xT_pool.tile([P, 8, 512], FP16, tag="xc", name=f"xv_{sc}")
            nc.sync.dma_start(
                out=xc,
                in_=xvT[:, 8 * 512 * sc:8 * 512 * (sc + 1)]
                    .rearrange("p (t s) -> p t s", s=512))
            xv_tiles[sc] = xc

        def emit_v_proj(sc):
            if sc not in xv_tiles:
                vp_dma(sc)
            xc = xv_tiles.pop(sc)
            for st in range(4):
                pv = psB.tile([P, DC], F32, tag="ps1", name=f"pv_{sc}_{st}")
                for dt in range(8):
                    nc.tensor.matmul(
                        pv,
                        lhsT=xc[:, dt, 128 * st:128 * (st + 1)],
                        rhs=w_v[:, dt, :],
                        start=(dt == 0), stop=(dt == 7))
                kt = 4 * sc + st
                nc.vector.tensor_copy(
                    v_sb[:, kt, :].rearrange("p (h c) -> p h c", h=H)[:, :, 0:DK],
                    pv.rearrange("p (h c) -> p h c", c=DK))

        pT_tiles = {}

        def emit_scores_exp(qc, pr_i, fillers=()):
            """Scores+exp for head pair pr_i over q-chunk qc.  `fillers` are
            emit-callbacks sprinkled between kg groups to keep the PE busy
            while ACT drains psA tiles."""
            fillers = list(fillers)
            qsl = slice(512 * qc, 512 * (qc + 1))
            h_a, h_b = 2 * pr_i, 2 * pr_i + 1
            pT_a = pT_pool.tile([P, 16, 512], FP16, tag="pT",
                                name=f"pT_{qc}_{h_a}")
            pT_b = pT_pool.tile([P, 16, 512], FP16, tag="pT",
                                name=f"pT_{qc}_{h_b}")
            pT_tiles[(qc, h_a)] = pT_a
            pT_tiles[(qc, h_b)] = pT_b
            for kg in range(8):
                sc_a = psA.tile([P, 2, 512], F32, tag="sc",
                                name=f"sca_{qc}_{pr_i}_{kg}")
                sc_b = psA.tile([P, 2, 512], F32, tag="sc",
                                name=f"scb_{qc}_{pr_i}_{kg}")
                # a-halves first: exp_a's input is complete one matmul
                # earlier, giving ACT a head start on draining psA
                for khi in range(2):
                    ksl = slice(128 * (2 * kg + khi), 128 * (2 * kg + khi + 1))
                    nc.tensor.matmul(sc_a[:, khi, :],
                                     lhsT=kT[0:64, pr_i, ksl],
                                     rhs=qT[0:64, pr_i, qsl])
                for khi in range(2):
                    ksl = slice(128 * (2 * kg + khi), 128 * (2 * kg + khi + 1))
                    nc.tensor.matmul(sc_b[:, khi, :],
                                     lhsT=kT[64:128, pr_i, ksl],
                                     rhs=qT[64:128, pr_i, qsl])
                for pt, sct in ((pT_a, sc_a), (pT_b, sc_b)):
                    dst = pt[:, 2 * kg:2 * kg + 2, :].rearrange(
                        "p a b -> p (a b)")
                    src = sct.rearrange("p a b -> p (a b)")
                    if kg in SCHR_KGS:
                        nc.vector.tensor_scalar(
                            out=dst.bitcast(I16), in0=src,
                            scalar1=SCHR_C1, scalar2=SCHR_C2,
                            op0=mybir.AluOpType.mult,
                            op1=mybir.AluOpType.add)
                    else:
                        nc.scalar.activation(
                            dst, src, mybir.ActivationFunctionType.Exp)
                if kg in (1, 3, 5) and fillers:
                    fillers.pop(0)()
            while fillers:
                fillers.pop(0)()

        def emit_ctx_norm(qc, h):
            qsl = slice(512 * qc, 512 * (qc + 1))
            pr_i, hp = divmod(h, 2)
            pT_h = pT_tiles.pop((qc, h))
            acc = psB.tile([P, 512], F32, tag="ps1", name=f"cp_{qc}_{h}")
            for kt in range(16):
                nc.tensor.matmul(
                    acc[0:65, :],
                    lhsT=v_sb[:, kt, 65 * h:65 * h + 65],
                    rhs=pT_h[:, kt, :],
                    start=(kt == 0), stop=(kt == 15))
            # normalize: ctx_n = ctx * broadcast(1/rowsum)
            rs = norm_pool.tile([1, 512], F32, tag="rs", name=f"rs_{qc}_{h}")
            nc.vector.tensor_copy(rs, acc[64:65, :])
            bc = norm_pool.tile([64, 512], F32, tag="bc", name=f"bc_{qc}_{h}")
            nc.gpsimd.partition_broadcast(bc, rs[0:1, :], channels=64)
            nc.vector.reciprocal_approx_fast(bc, bc)
            nc.vector.tensor_mul(
                ctxn[64 * hp:64 * hp + 64, pr_i, qsl], acc[0:64, :], bc)

        def emit_outproj_st(st, split_dma=False):
            ob = out_sb_pool.tile([P, D], FP16, tag="ob", name=f"ob_{st}")
            for jc in range(2):
                op = psB.tile([P, 512], F32, tag="ps1", name=f"op_{st}_{jc}")
                for ct in range(2):
                    nc.tensor.matmul(
                        op,
                        lhsT=ctxn[:, ct, 128 * st:128 * (st + 1)],
                        rhs=w_o[:, ct, 512 * jc:512 * (jc + 1)],
                        start=(ct == 0), stop=(ct == 1))
                nc.vector.tensor_copy(ob[:, 512 * jc:512 * (jc + 1)], op)
                if split_dma:  # final sts: post each half ASAP (tail latency)
                    nc.sync.dma_start(
                        out=out[128 * st:128 * (st + 1),
                                512 * jc:512 * (jc + 1)],
                        in_=ob[:, 512 * jc:512 * (jc + 1)])
            if not split_dma:  # one post per st: 128 contiguous 2KB rows
                nc.sync.dma_start(out=out[128 * st:128 * (st + 1), :], in_=ob)

        # --- emission schedule (software pipeline) --------------------------------
        def ctx_(qc, h):
            return lambda: emit_ctx_norm(qc, h)

        def op(st):
            return lambda: emit_outproj_st(st)

        def vp(sc):
            return lambda: emit_v_proj(sc)

        def chunk_dma(x_dram, c0, cols, name):
            xc = xT_pool.tile([P, 8, cols], FP16, tag="xc",
                              name=f"xh_{name}{c0}")
            nc.sync.dma_start(
                out=xc,
                in_=x_dram[:, 8 * c0:8 * (c0 + cols)]
                    .rearrange("p (t s) -> p t s", s=cols))
            return xc

        def emit_qk_proj_chunk(name, xc, w_t, b_t, dest, c0, cols):
            for m in range(2):
                pr = psB.tile([P, cols], F32, tag="ps1",
                              name=f"prh_{name}_{c0}_{m}")
                for dt in range(8):
                    nc.tensor.matmul(
                        pr,
                        lhsT=w_t[:, dt, 128 * m:128 * (m + 1)],
                        rhs=xc[:, dt, :],
                        start=(dt == 0), stop=(dt == 7))
                nc.vector.tensor_scalar_add(
                    dest[:, m, c0:c0 + cols], pr, b_t[:, m:m + 1])

        # post order is emission order on the Sync engine; every post is
        # emitted BEFORE any instruction that reads its tile (Tile deps are
        # emission-order-based), with the critical first chunks up front.
        xkA = chunk_dma(xkT, 0, 128, "k")
        nc.sync.dma_start(out=w_k[:, 4:8, :], in_=wk_r[:, 4:8, :])
        nc.sync.dma_start(out=bk_t, in_=bkr)
        xkB = chunk_dma(xkT, 128, 128, "k")
        emit_qk_proj_chunk("k", xkA, w_k, bk_t, kT, 0, 128)
        xkC = chunk_dma(xkT, 256, 256, "k")
        emit_qk_proj_chunk("k", xkB, w_k, bk_t, kT, 128, 128)
        emit_qk_proj_chunk("k", xkC, w_k, bk_t, kT, 256, 256)
        for sc in range(1, 4):
            emit_qk_proj("k", xkT, w_k, bk_t, kT, sc)

        w_q = singles.tile([P, 8, DC], FP16, tag="w_q")
        bq_t = singles.tile([P, 2], F32, tag="bq")
        nc.sync.dma_start(out=w_q, in_=wqt.rearrange("p (t c) -> p t c", c=DC))
        nc.sync.dma_start(out=bq_t, in_=bqr)
        emit_qk_proj("q", xqT, w_q, bq_t, qT, 0)

        w_v = singles.tile([P, 8, DC], FP16, tag="w_v")
        nc.sync.dma_start(out=w_v, in_=wvt.rearrange("p (t c) -> p t c", c=DC))
        vp_dma(0)
        vp_dma(1)
        for h in range(H):  # ones column per head for rowsum-in-matmul
            nc.vector.memset(v_sb[:, :, h * 65 + 64:h * 65 + 65], 1.0)
        w_o = singles.tile([P, 2, D], FP16, tag="w_o")
        nc.sync.dma_start(out=w_o, in_=wot.rearrange("p (t j) -> p t j", j=D))

        xq_tiles = {}

        def qp_dma(sc):
            xc = xT_pool.tile([P, 8, 512], FP16, tag="xc", name=f"xq_{sc}")
            nc.sync.dma_start(
                out=xc,
                in_=xqT[:, 8 * 512 * sc:8 * 512 * (sc + 1)]
                    .rearrange("p (t s) -> p t s", s=512))
            xq_tiles[sc] = xc

        def emit_q_proj(sc):
            emit_qk_proj("q", xqT, w_q, bq_t, qT, sc, xc=xq_tiles.pop(sc))

        def qp(sc):
            return lambda: emit_q_proj(sc)

        # pT pool holds 7 tiles; a scores call allocates 2, so enough ctx
        # consumers must be emitted before the scores call that recycles
        # their buffers (live pT <= 5 at every scores entry).
        # x-chunk DMAs are prefetched 1+ scores-call ahead so filler
        # projections never wait on an in-flight post.
        emit_scores_exp(0, 0, fillers=[vp(0), vp(1)])
        vp_dma(2)
        vp_dma(3)
        qp_dma(1)
        emit_scores_exp(0, 1, fillers=[vp(2), vp(3)])
        emit_q_proj(1)
        qp_dma(2)
        emit_scores_exp(1, 0, fillers=[qp(2)])
        qp_dma(3)
        emit_ctx_norm(0, 0)
        emit_ctx_norm(0, 1)
        emit_scores_exp(1, 1, fillers=[qp(3), ctx_(0, 2)])
        emit_scores_exp(2, 0, fillers=[ctx_(0, 3), op(0), op(1)])
        emit_ctx_norm(1, 0)
        emit_ctx_norm(1, 1)
        emit_scores_exp(2, 1, fillers=[op(2), op(3), ctx_(1, 2)])
        emit_scores_exp(3, 0, fillers=[ctx_(1, 3), op(4), op(5)])
        emit_ctx_norm(2, 0)
        emit_ctx_norm(2, 1)
        # tail: interleave qc3 ctx with qc2 out-proj so no op waits on a
        # just-emitted norm chain (psB recycle), and keep the PE stream dense
        emit_scores_exp(3, 1, fillers=[op(6), op(7), ctx_(2, 2), ctx_(2, 3),
                                       op(8), op(9), ctx_(3, 0), op(10),
                                       ctx_(3, 1), op(11)])
        emit_ctx_norm(3, 2)
        emit_ctx_norm(3, 3)
        for st in range(12, 16):
            emit_outproj_st(st, split_dma=(st >= 14))


def _stage_x(xT, widths):
    """xT [D, S] -> [128, 8*S]: consumption-ordered col blocks, each stored
    p-major so every chunk DMA is 128 contiguous per-partition rows."""
    blocks = []
    c0 = 0
    for w in widths:
        blk = xT[:, c0:c0 + w].reshape(8, P, w).transpose(1, 0, 2)
        blocks.append(blk.reshape(P, 8 * w))
        c0 += w
    return np.ascontiguousarray(np.concatenate(blocks, axis=1))


def _stage_w(wt):
    """wt [D, DC] -> [128, 8*DC] p-major (tile layout [P, 8, DC] flattened)."""
    return np.ascontiguousarray(
        wt.reshape(8, P, DC).transpose(1, 0, 2).reshape(P, 8 * DC))


K_WIDTHS = (128, 128, 256, 512, 512, 512)
QV_WIDTHS = (512, 512, 512, 512)


def make_in_maps(Q_input, K_input, V_input, Wq, bq, Wk, bk, Wv, Wo):
    scale = 0.125  # 1/sqrt(64), exact power of two
    xS = {}
    for b in range(2):
        xS[("q", b)] = _stage_x(Q_input[b].T.astype(np.float16), QV_WIDTHS)
        xS[("k", b)] = _stage_x(K_input[b].T.astype(np.float16), K_WIDTHS)
        xS[("v", b)] = _stage_x(V_input[b].T.astype(np.float16), QV_WIDTHS)
    in_maps = []
    for c in range(NCORES):
        b, g = divmod(c, 4)
        sl = slice(DC * g, DC * (g + 1))
        wo_p = np.ascontiguousarray(
            Wo[:, sl].T.reshape(2, P, D).transpose(1, 0, 2).reshape(P, 2 * D))
        in_maps.append({
            "xqT": xS[("q", b)],
            "xkT": xS[("k", b)],
            "xvT": xS[("v", b)],
            "wqt": _stage_w((Wq[sl, :].T * scale).astype(np.float16)),
            "wkt": _stage_w(Wk[sl, :].T.astype(np.float16)),
            "wvt": _stage_w(Wv[sl, :].T.astype(np.float16)),
            "wot": wo_p.astype(np.float16),
            "bqr": (bq[sl] * scale).reshape(2, P).T.astype(np.float32),
            "bkr": bk[sl].reshape(2, P).T.astype(np.float32),
        })
    return in_maps


def kernel(Q_input, K_input, V_input, Wq, bq, Wk, bk, Wv, bv, Wo, bo):
    if "nc" not in _cached:
        _cached["nc"] = build_program()
    nc = _cached["nc"]

    in_maps = make_in_maps(Q_input, K_input, V_input, Wq, bq, Wk, bk, Wv, Wo)
    res = run_bass_kernel_spmd(nc, in_maps, list(range(NCORES))).results
    outs = [res[c]["out"] for c in range(NCORES)]

    const = (bv.astype(np.float32) @ Wo.T.astype(np.float32)) + bo
    full = np.empty((2, S, D), np.float32)
    for b in range(2):
        acc = outs[4 * b].astype(np.float32)
        for g in range(1, 4):
            acc += outs[4 * b + g]
        full[b] = acc + const
    return full



# revision 36
# speedup vs baseline: 1.0350x; 1.0022x over previous
"""MultiHeadAttention Trainium2 kernel (8 NeuronCores).

Sharding: data-parallel over batch (2) x tensor-parallel over heads (16/4=4
head groups). Core c handles batch b = c//4 and heads 4g..4g+4 (g = c%4),
i.e. a 256-wide column slice of Wq/Wk/Wv and the matching row slice of Wo.
Each core computes a full [2048, 1024] partial output (its heads' ctx @ Wo
row-slice); the host sums the 4 partials per batch and adds the bias terms.

v4: fp16 on-chip datapath.  (fp8 Q/K + DoubleRow projections were built
and measured but rejected: e4m3 score noise passes through softmax to ctx
proportionally, ~9% max-err vs the 2e-2 gate.  fp8 DoublePixel scores are
numerically exact but run at half rate on this silicon.  A DVE Schraudolph
exp offload works (SCHR_KGS) but couples the DVE queue into the psA
critical path for no net gain, so it is off.)
On top of the v2 pipeline:
 - fp16 partial-output DMA (host sums partials in f32): halves out traffic.
 - head: K weights+bias DMA first, K chunk DMAs immediately after, other
   weights deferred -> first matmul starts ~11us in (was ~21us).
 - fine-grained schedule: V-proj/Q-proj/ctx/out-proj chains are emitted
   between scores kg-groups (PE program order = emission order), keeping
   the PE stream dense (>92% busy) while ACT drains psA score tiles.
 - V-chunk DMA prefetch ahead of the V-projection fillers; first K
   chunk split in two so the first matmul waits on only 0.75 MB of DMA;
   out-proj halves DMA'd as soon as each copy lands (shorter drain).
 - pT pool discipline: a scores call allocates 2 of 7 pT buffers; enough
   ctx consumers are emitted before the scores call that recycles their
   buffers (live pT <= 5 at every scores entry).
Measured: ~228us HW exec (baseline v2: ~243-250us), rel err 8.5e-4.
"""

import numpy as np

import concourse.bass as bass
import concourse.mybir as mybir
import concourse.tile as tile
from concourse import bacc
from concourse.bass_utils import run_bass_kernel_spmd

S = 2048          # sequence length
D = 1024          # model dim
DC = 256          # d' columns per core (4 heads x 64)
H = 4             # heads per core
DK = 64           # head dim
P = 128
F32 = mybir.dt.float32
FP16 = mybir.dt.float16
I16 = mybir.dt.int16
NCORES = 8

SCHR_KGS = (0, 4)  # kg tiles whose exp runs on DVE (Schraudolph)
WARMUP_MMS = 34   # dep-free 128-row matmuls that ramp the PE clock at t=0

# Schraudolph fp16-bit exp constants (scores arrive pre-scaled: the 0.125
# softmax scale is folded into Wq/bq on the host)
SCHR_C1 = 1024.0 * 1.4426950408889634
SCHR_C2 = 15360.0 - 46.0

_cached = {}


def build_program():
    nc = bacc.Bacc("TRN2", target_bir_lowering=False, debug=False,
                   num_devices=NCORES)

    # all staged inputs are host-pre-arranged so every DMA is one post with
    # per-partition-contiguous rows (128 fat descriptors, not 1024 thin ones)
    xqT = nc.dram_tensor("xqT", [P, 8 * S], FP16, kind="ExternalInput").ap()
    xkT = nc.dram_tensor("xkT", [P, 8 * S], FP16, kind="ExternalInput").ap()
    xvT = nc.dram_tensor("xvT", [P, 8 * S], FP16, kind="ExternalInput").ap()
    wqt = nc.dram_tensor("wqt", [P, 8 * DC], FP16, kind="ExternalInput").ap()
    wkt = nc.dram_tensor("wkt", [P, 8 * DC], FP16, kind="ExternalInput").ap()
    wvt = nc.dram_tensor("wvt", [P, 8 * DC], FP16, kind="ExternalInput").ap()
    wot = nc.dram_tensor("wot", [P, 2 * D], FP16, kind="ExternalInput").ap()
    bqr = nc.dram_tensor("bqr", [P, 2], F32, kind="ExternalInput").ap()
    bkr = nc.dram_tensor("bkr", [P, 2], F32, kind="ExternalInput").ap()
    out = nc.dram_tensor("out", [S, D], FP16, kind="ExternalOutput").ap()

    with tile.TileContext(nc) as tc:
        build_tile_kernel(nc, tc, xqT, xkT, xvT, wqt, wkt, wvt, wot,
                          bqr, bkr, out)

    nc.compile()
    return nc


def build_tile_kernel(nc, tc, xqT, xkT, xvT, wqt, wkt, wvt, wot,
                      bqr, bkr, out):
    from contextlib import ExitStack

    with ExitStack() as ctx:
        singles = ctx.enter_context(tc.tile_pool(name="singles", bufs=1))
        persist = ctx.enter_context(tc.tile_pool(name="persist", bufs=1))
        psA = ctx.enter_context(tc.tile_pool(name="psA", bufs=2, space="PSUM"))
        psB = ctx.enter_context(tc.tile_pool(name="psB", bufs=4, space="PSUM"))
        xT_pool = ctx.enter_context(tc.tile_pool(name="xT", bufs=4))
        pT_pool = ctx.enter_context(tc.tile_pool(name="pT", bufs=7))
        norm_pool = ctx.enter_context(tc.tile_pool(name="norm", bufs=2))
        out_sb_pool = ctx.enter_context(tc.tile_pool(name="osb", bufs=2))

        # --- PE warmup: dep-free dummy matmuls ramp the clock out of its low
        # p-state while the head DMAs land (real matmuls then start hot) -------
        warm = singles.tile([P, 128], FP16, tag="warm")
        nc.vector.memset(warm, 0.0)
        warm_ps = psB.tile([P, 128], F32, tag="ps1", name="warm_ps")
        for _ in range(WARMUP_MMS):
            nc.tensor.matmul(warm_ps, lhsT=warm, rhs=warm)

        # --- weights (K first so the K projection can start ASAP) ---------------
        w_k = singles.tile([P, 8, DC], FP16, tag="w_k")
        bk_t = singles.tile([P, 2], F32, tag="bk")
        # two posts: first matmul needs only dt 0-3; dt 4-7 drain behind it
        wk_r = wkt.rearrange("p (t c) -> p t c", c=DC)
        nc.sync.dma_start(out=w_k[:, 0:4, :], in_=wk_r[:, 0:4, :])

        # --- persistent activations ---------------------------------------------
        qT = persist.tile([P, 2, S], FP16, tag="qT")    # [d'%128, pair, s]
        kT = persist.tile([P, 2, S], FP16, tag="kT")
        v_sb = persist.tile([P, 16, H * (DK + 1)], FP16, tag="v_sb")
        ctxn = persist.tile([P, 2, S], FP16, tag="ctxn")  # [c%128, pair, q]

        # --- emit helpers ---------------------------------------------------------
        def emit_qk_proj(name, x_dram, w_t, b_t, dest, sc, xc=None):
            if xc is None:
                xc = xT_pool.tile([P, 8, 512], FP16, tag="xc",
                                  name=f"xc_{name}{sc}")
                nc.sync.dma_start(
                    out=xc,
                    in_=x_dram[:, 8 * 512 * sc:8 * 512 * (sc + 1)]
                        .rearrange("p (t s) -> p t s", s=512))
            for m in range(2):
                pr = psB.tile([P, 512], F32, tag="ps1",
                              name=f"pr_{name}_{sc}_{m}")
                for dt in range(8):
                    nc.tensor.matmul(
                        pr,
                        lhsT=w_t[:, dt, 128 * m:128 * (m + 1)],
                        rhs=xc[:, dt, :],
                        start=(dt == 0), stop=(dt == 7))
                nc.vector.tensor_scalar_add(
                    dest[:, m, 512 * sc:512 * (sc + 1)], pr, b_t[:, m:m + 1])

        xv_tiles = {}

        def vp_dma(sc):
            xc = xT_pool.tile([P, 8, 512], FP16, tag="xc", name=f"xv_{sc}")
            nc.sync.dma_start(
                out=xc,
                in_=xvT[:, 8 * 512 * sc:8 * 512 * (sc + 1)]
                    .rearrange("p (t s) -> p t s", s=512))
            xv_tiles[sc] = xc

        def emit_v_proj(sc):
            if sc not in xv_tiles:
                vp_dma(sc)
            xc = xv_tiles.pop(sc)
            for st in range(4):
                pv = psB.tile([P, DC], F32, tag="ps1", name=f"pv_{sc}_{st}")
                for dt in range(8):
                    nc.tensor.matmul(
                        pv,
                        lhsT=xc[:, dt, 128 * st:128 * (st + 1)],
                        rhs=w_v[:, dt, :],
                        start=(dt == 0), stop=(dt == 7))
                kt = 4 * sc + st
                nc.vector.tensor_copy(
                    v_sb[:, kt, :].rearrange("p (h c) -> p h c", h=H)[:, :, 0:DK],
                    pv.rearrange("p (h c) -> p h c", c=DK))

        pT_tiles = {}

        def emit_scores_exp(qc, pr_i, fillers=()):
            """Scores+exp for head pair pr_i over q-chunk qc.  `fillers` are
            emit-callbacks sprinkled between kg groups to keep the PE busy
            while ACT drains psA tiles."""
            fillers = list(fillers)
            qsl = slice(512 * qc, 512 * (qc + 1))
            h_a, h_b = 2 * pr_i, 2 * pr_i + 1
            pT_a = pT_pool.tile([P, 16, 512], FP16, tag="pT",
                                name=f"pT_{qc}_{h_a}")
            pT_b = pT_pool.tile([P, 16, 512], FP16, tag="pT",
                                name=f"pT_{qc}_{h_b}")
            pT_tiles[(qc, h_a)] = pT_a
            pT_tiles[(qc, h_b)] = pT_b
            for kg in range(8):
                sc_a = psA.tile([P, 2, 512], F32, tag="sc",
                                name=f"sca_{qc}_{pr_i}_{kg}")
                sc_b = psA.tile([P, 2, 512], F32, tag="sc",
                                name=f"scb_{qc}_{pr_i}_{kg}")
                # a-halves first: exp_a's input is complete one matmul
                # earlier, giving ACT a head start on draining psA
                for khi in range(2):
                    ksl = slice(128 * (2 * kg + khi), 128 * (2 * kg + khi + 1))
                    nc.tensor.matmul(sc_a[:, khi, :],
                                     lhsT=kT[0:64, pr_i, ksl],
                                     rhs=qT[0:64, pr_i, qsl])
                for khi in range(2):
                    ksl = slice(128 * (2 * kg + khi), 128 * (2 * kg + khi + 1))
                    nc.tensor.matmul(sc_b[:, khi, :],
                                     lhsT=kT[64:128, pr_i, ksl],
                                     rhs=qT[64:128, pr_i, qsl])
                for pt, sct in ((pT_a, sc_a), (pT_b, sc_b)):
                    dst = pt[:, 2 * kg:2 * kg + 2, :].rearrange(
                        "p a b -> p (a b)")
                    src = sct.rearrange("p a b -> p (a b)")
                    if kg in SCHR_KGS:
                        nc.vector.tensor_scalar(
                            out=dst.bitcast(I16), in0=src,
                            scalar1=SCHR_C1, scalar2=SCHR_C2,
                            op0=mybir.AluOpType.mult,
                            op1=mybir.AluOpType.add)
                    else:
                        nc.scalar.activation(
                            dst, src, mybir.ActivationFunctionType.Exp)
                if kg in (1, 3, 5) and fillers:
                    fillers.pop(0)()
            while fillers:
                fillers.pop(0)()

        def emit_ctx_norm(qc, h):
            qsl = slice(512 * qc, 512 * (qc + 1))
            pr_i, hp = divmod(h, 2)
            pT_h = pT_tiles.pop((qc, h))
            acc = psB.tile([P, 512], F32, tag="ps1", name=f"cp_{qc}_{h}")
            for kt in range(16):
                nc.tensor.matmul(
                    acc[0:65, :],
                    lhsT=v_sb[:, kt, 65 * h:65 * h + 65],
                    rhs=pT_h[:, kt, :],
                    start=(kt == 0), stop=(kt == 15))
            # normalize: ctx_n = ctx * broadcast(1/rowsum)
            rs = norm_pool.tile([1, 512], F32, tag="rs", name=f"rs_{qc}_{h}")
            nc.vector.tensor_copy(rs, acc[64:65, :])
            bc = norm_pool.tile([64, 512], F32, tag="bc", name=f"bc_{qc}_{h}")
            nc.gpsimd.partition_broadcast(bc, rs[0:1, :], channels=64)
            nc.vector.reciprocal_approx_fast(bc, bc)
            nc.vector.tensor_mul(
                ctxn[64 * hp:64 * hp + 64, pr_i, qsl], acc[0:64, :], bc)

        def emit_outproj_st(st, split_dma=False):
            ob = out_sb_pool.tile([P, D], FP16, tag="ob", name=f"ob_{st}")
            for jc in range(2):
                op = psB.tile([P, 512], F32, tag="ps1", name=f"op_{st}_{jc}")
                for ct in range(2):
                    nc.tensor.matmul(
                        op,
                        lhsT=ctxn[:, ct, 128 * st:128 * (st + 1)],
                        rhs=w_o[:, ct, 512 * jc:512 * (jc + 1)],
                        start=(ct == 0), stop=(ct == 1))
                nc.vector.tensor_copy(ob[:, 512 * jc:512 * (jc + 1)], op)
                if split_dma:  # final sts: post each half ASAP (tail latency)
                    nc.sync.dma_start(
                        out=out[128 * st:128 * (st + 1),
                                512 * jc:512 * (jc + 1)],
                        in_=ob[:, 512 * jc:512 * (jc + 1)])
            if not split_dma:  # one post per st: 128 contiguous 2KB rows
                nc.sync.dma_start(out=out[128 * st:128 * (st + 1), :], in_=ob)

        # --- emission schedule (software pipeline) --------------------------------
        def ctx_(qc, h):
            return lambda: emit_ctx_norm(qc, h)

        def op(st):
            return lambda: emit_outproj_st(st)

        def vp(sc):
            return lambda: emit_v_proj(sc)

        def chunk_dma(x_dram, c0, cols, name):
            xc = xT_pool.tile([P, 8, cols], FP16, tag="xc",
                              name=f"xh_{name}{c0}")
            nc.sync.dma_start(
                out=xc,
                in_=x_dram[:, 8 * c0:8 * (c0 + cols)]
                    .rearrange("p (t s) -> p t s", s=cols))
            return xc

        def emit_qk_proj_chunk(name, xc, w_t, b_t, dest, c0, cols):
            for m in range(2):
                pr = psB.tile([P, cols], F32, tag="ps1",
                              name=f"prh_{name}_{c0}_{m}")
                for dt in range(8):
                    nc.tensor.matmul(
                        pr,
                        lhsT=w_t[:, dt, 128 * m:128 * (m + 1)],
                        rhs=xc[:, dt, :],
                        start=(dt == 0), stop=(dt == 7))
                nc.vector.tensor_scalar_add(
                    dest[:, m, c0:c0 + cols], pr, b_t[:, m:m + 1])

        # post order is emission order on the Sync engine; every post is
        # emitted BEFORE any instruction that reads its tile (Tile deps are
        # emission-order-based), with the critical first chunks up front.
        xkA = chunk_dma(xkT, 0, 128, "k")
        nc.sync.dma_start(out=w_k[:, 4:8, :], in_=wk_r[:, 4:8, :])
        nc.sync.dma_start(out=bk_t, in_=bkr)
        xkB = chunk_dma(xkT, 128, 128, "k")
        emit_qk_proj_chunk("k", xkA, w_k, bk_t, kT, 0, 128)
        xkC = chunk_dma(xkT, 256, 256, "k")
        emit_qk_proj_chunk("k", xkB, w_k, bk_t, kT, 128, 128)
        emit_qk_proj_chunk("k", xkC, w_k, bk_t, kT, 256, 256)
        for sc in range(1, 4):
            emit_qk_proj("k", xkT, w_k, bk_t, kT, sc)

        w_q = singles.tile([P, 8, DC], FP16, tag="w_q")
        bq_t = singles.tile([P, 2], F32, tag="bq")
        nc.sync.dma_start(out=w_q, in_=wqt.rearrange("p (t c) -> p t c", c=DC))
        nc.sync.dma_start(out=bq_t, in_=bqr)
        emit_qk_proj("q", xqT, w_q, bq_t, qT, 0)

        w_v = singles.tile([P, 8, DC], FP16, tag="w_v")
        nc.sync.dma_start(out=w_v, in_=wvt.rearrange("p (t c) -> p t c", c=DC))
        vp_dma(0)
        vp_dma(1)
        for h in range(H):  # ones column per head for rowsum-in-matmul
            nc.vector.memset(v_sb[:, :, h * 65 + 64:h * 65 + 65], 1.0)
        w_o = singles.tile([P, 2, D], FP16, tag="w_o")
        nc.sync.dma_start(out=w_o, in_=wot.rearrange("p (t j) -> p t j", j=D))

        xq_tiles = {}

        def qp_dma(sc):
            xc = xT_pool.tile([P, 8, 512], FP16, tag="xc", name=f"xq_{sc}")
            nc.sync.dma_start(
                out=xc,
                in_=xqT[:, 8 * 512 * sc:8 * 512 * (sc + 1)]
                    .rearrange("p (t s) -> p t s", s=512))
            xq_tiles[sc] = xc

        def emit_q_proj(sc):
            emit_qk_proj("q", xqT, w_q, bq_t, qT, sc, xc=xq_tiles.pop(sc))

        def qp(sc):
            return lambda: emit_q_proj(sc)

        # pT pool holds 7 tiles; a scores call allocates 2, so enough ctx
        # consumers must be emitted before the scores call that recycles
        # their buffers (live pT <= 5 at every scores entry).
        # x-chunk DMAs are prefetched 1+ scores-call ahead so filler
        # projections never wait on an in-flight post.
        emit_scores_exp(0, 0, fillers=[vp(0), vp(1)])
        vp_dma(2)
        vp_dma(3)
        qp_dma(1)
        emit_scores_exp(0, 1, fillers=[vp(2), vp(3)])
        emit_q_proj(1)
        qp_dma(2)
        emit_scores_exp(1, 0, fillers=[qp(2)])
        qp_dma(3)
        emit_ctx_norm(0, 0)
        emit_ctx_norm(0, 1)
        emit_scores_exp(1, 1, fillers=[qp(3), ctx_(0, 2)])
        emit_scores_exp(2, 0, fillers=[ctx_(0, 3), op(0), op(1)])
        emit_ctx_norm(1, 0)
        emit_ctx_norm(1, 1)
        emit_scores_exp(2, 1, fillers=[op(2), op(3), ctx_(1, 2)])
        emit_scores_exp(3, 0, fillers=[ctx_(1, 3), op(4), op(5)])
        emit_ctx_norm(2, 0)
        emit_ctx_norm(2, 1)
        # tail: interleave qc3 ctx with qc2 out-proj so no op waits on a
        # just-emitted norm chain (psB recycle), and keep the PE stream dense
        emit_scores_exp(3, 1, fillers=[op(6), op(7), ctx_(2, 2), ctx_(2, 3),
                                       op(8), op(9), ctx_(3, 0), op(10),
                                       ctx_(3, 1), op(11)])
        emit_ctx_norm(3, 2)
        emit_ctx_norm(3, 3)
        for st in range(12, 16):
            emit_outproj_st(st, split_dma=(st >= 14))


def _stage_x(xT, widths):
    """xT [D, S] -> [128, 8*S]: consumption-ordered col blocks, each stored
    p-major so every chunk DMA is 128 contiguous per-partition rows."""
    blocks = []
    c0 = 0
    for w in widths:
        blk = xT[:, c0:c0 + w].reshape(8, P, w).transpose(1, 0, 2)
        blocks.append(blk.reshape(P, 8 * w))
        c0 += w
    return np.ascontiguousarray(np.concatenate(blocks, axis=1))


def _stage_w(wt):
    """wt [D, DC] -> [128, 8*DC] p-major (tile layout [P, 8, DC] flattened)."""
    return np.ascontiguousarray(
        wt.reshape(8, P, DC).transpose(1, 0, 2).reshape(P, 8 * DC))


K_WIDTHS = (128, 128, 256, 512, 512, 512)
QV_WIDTHS = (512, 512, 512, 512)


def make_in_maps(Q_input, K_input, V_input, Wq, bq, Wk, bk, Wv, Wo):
    scale = 0.125  # 1/sqrt(64), exact power of two
    xS = {}
    for b in range(2):
        xS[("q", b)] = _stage_x(Q_input[b].T.astype(np.float16), QV_WIDTHS)
        xS[("k", b)] = _stage_x(K_input[b].T.astype(np.float16), K_WIDTHS)
        xS[("v", b)] = _stage_x(V_input[b].T.astype(np.float16), QV_WIDTHS)
    in_maps = []
    for c in range(NCORES):
        b, g = divmod(c, 4)
        sl = slice(DC * g, DC * (g + 1))
        wo_p = np.ascontiguousarray(
            Wo[:, sl].T.reshape(2, P, D).transpose(1, 0, 2).reshape(P, 2 * D))
        in_maps.append({
            "xqT": xS[("q", b)],
            "xkT": xS[("k", b)],
            "xvT": xS[("v", b)],
            "wqt": _stage_w((Wq[sl, :].T * scale).astype(np.float16)),
            "wkt": _stage_w(Wk[sl, :].T.astype(np.float16)),
            "wvt": _stage_w(Wv[sl, :].T.astype(np.float16)),
            "wot": wo_p.astype(np.float16),
            "bqr": (bq[sl] * scale).reshape(2, P).T.astype(np.float32),
            "bkr": bk[sl].reshape(2, P).T.astype(np.float32),
        })
    return in_maps


def kernel(Q_input, K_input, V_input, Wq, bq, Wk, bk, Wv, bv, Wo, bo):
    if "nc" not in _cached:
        _cached["nc"] = build_program()
    nc = _cached["nc"]

    in_maps = make_in_maps(Q_input, K_input, V_input, Wq, bq, Wk, bk, Wv, Wo)
    res = run_bass_kernel_spmd(nc, in_maps, list(range(NCORES))).results
    outs = [res[c]["out"] for c in range(NCORES)]

    const = (bv.astype(np.float32) @ Wo.T.astype(np.float32)) + bo
    full = np.empty((2, S, D), np.float32)
    for b in range(2):
        acc = outs[4 * b].astype(np.float32)
        for g in range(1, 4):
            acc += outs[4 * b + g]
        full[b] = acc + const
    return full

